# revision 9
# baseline (speedup 1.0000x reference)
"""Trainium2 kernel for nn_CompositeFullyConnected (MoE-style blocked MLP).

Reference computes, per sample b:
    h = relu(x @ W1 + b1); h = relu(h @ W2 + b2)
    h = relu(h @ Kb1[:,:,k] + Bb1[:,:,k]);  out = h @ Kb2[:,:,k] + Bb2[:,:,k]
where k = states[b].  The reference evaluates ALL 16 expert blocks and then
selects one; only the selected block's path is actually needed.

Strategy (all routing on host, static shapes on device):
  - Sort samples by state.  Assign states (2c, 2c+1) to core c; pad each
    state's group to SCAP rows (zeros).  Each core processes R = 2*SCAP rows.
  - Activations live transposed ([features, rows]) so features sit on SBUF
    partitions; weights are naturally [in, out] = lhsT.  No device transposes:
    the host ships x pre-transposed and transposes the output back.
  - All weights + activations fit in SBUF; one DMA load each, fp32r matmuls
    (full PE rate at free-dim >= 256), bias+relu fused on the scalar engine.
"""

from contextlib import ExitStack

import numpy as np

import concourse.bass as bass
import concourse.mybir as mybir
import concourse.tile as tile
from concourse import bacc
from concourse.bass import ts
from concourse.bass_utils import run_bass_kernel_spmd

P = 128
B, F = 4096, 512
H1, H2, U1, U2 = 1024, 1024, 512, 256
K = 16
NCORES = 8
SCAP = 320           # per-state row capacity (seed-0 max count is 275)
R = 2 * SCAP         # rows per core
FREE = SCAP          # matmul free-dim chunk == one state's rows
FP = mybir.dt.float32
FPR = mybir.dt.float32r  # matmul operand storage (fp32 bytes, PE rounds)

# bias SBUF column layout: [b1(8) | b2(8) | Bb1 s0(4) s1(4) | Bb2 s0(2) s1(2)]
_B1_COL, _B2_COL, _BB1_COL, _BB2_COL = 0, 8, 16, 24
_NBIAS = 28


def _body(tc, ctx):
    nc = tc.nc
    xT = nc.dram_tensor("xT", [F, R], FPR, kind="ExternalInput")
    w1 = nc.dram_tensor("w1", [F, H1], FPR, kind="ExternalInput")
    w2 = nc.dram_tensor("w2", [H1, H2], FPR, kind="ExternalInput")
    kb1 = nc.dram_tensor("kb1", [2, H2, U1], FPR, kind="ExternalInput")
    kb2 = nc.dram_tensor("kb2", [2, U1, U2], FPR, kind="ExternalInput")
    biases = nc.dram_tensor("biases", [P, _NBIAS], FP, kind="ExternalInput")
    out = nc.dram_tensor("out", [U2, R], FP, kind="ExternalOutput")

    wpool = ctx.enter_context(tc.tile_pool(name="weights", bufs=1))
    apool = ctx.enter_context(tc.tile_pool(name="acts", bufs=1))
    pp = ctx.enter_context(tc.tile_pool(name="psum", bufs=8, space="PSUM"))

    relu = mybir.ActivationFunctionType.Relu
    ident = mybir.ActivationFunctionType.Identity

    # ---- loads, in order of first use ----
    bias_sb = wpool.tile([P, _NBIAS], FP, name="bias_sb", tag="bias_sb")
    nc.sync.dma_start(bias_sb[:], biases[:])
    x_sb = []
    for i in range(F // P):
        t = wpool.tile([P, R], FPR, name=f"x{i}", tag=f"x{i}")
        nc.sync.dma_start(t[:], xT[ts(i, P), :])
        x_sb.append(t)
    w1_sb = []
    for i in range(F // P):
        t = wpool.tile([P, H1], FPR, name=f"w1_{i}", tag=f"w1_{i}")
        nc.sync.dma_start(t[:], w1[ts(i, P), :])
        w1_sb.append(t)
    w2_sb = []
    for i in range(H1 // P):
        t = wpool.tile([P, H2], FPR, name=f"w2_{i}", tag=f"w2_{i}")
        nc.sync.dma_start(t[:], w2[ts(i, P), :])
        w2_sb.append(t)
    kb1_sb = [[], []]
    for s in range(2):
        for i in range(H2 // P):
            t = wpool.tile([P, U1], FPR, name=f"kb1_{s}_{i}", tag=f"kb1_{s}_{i}")
            nc.sync.dma_start(t[:], kb1[s, ts(i, P), :])
            kb1_sb[s].append(t)
    kb2_sb = [[], []]
    for s in range(2):
        for i in range(U1 // P):
            t = wpool.tile([P, U2], FPR, name=f"kb2_{s}_{i}", tag=f"kb2_{s}_{i}")
            nc.sync.dma_start(t[:], kb2[s, ts(i, P), :])
            kb2_sb[s].append(t)

    h1_sb = [apool.tile([P, R], FPR, name=f"h1_{m}", tag=f"h1_{m}") for m in range(H1 // P)]
    h2_sb = [apool.tile([P, R], FPR, name=f"h2_{m}", tag=f"h2_{m}") for m in range(H2 // P)]
    h3_sb = [apool.tile([P, R], FPR, name=f"h3_{m}", tag=f"h3_{m}") for m in range(U1 // P)]
    out_sb = [apool.tile([P, R], FP, name=f"o_{m}", tag=f"o_{m}") for m in range(U2 // P)]

    def mm_layer(dst_tiles, lhs_tiles, rhs_tiles, free_slices, bias_col_of, func):
        """dst[m][:, fs] = func(sum_k lhs[k][:, m*128:...]^T @ rhs[k][:, fs] + bias)"""
        nk = len(lhs_tiles)
        for m in range(len(dst_tiles)):
            for fi, fs in enumerate(free_slices):
                ps = pp.tile([P, FREE], mybir.dt.float32, name="ps", tag="ps")
                for k in range(nk):
                    nc.tensor.matmul(
                        ps[:],
                        lhs_tiles[k][:, ts(m, P)],
                        rhs_tiles[k][:, fs],
                        start=(k == 0),
                        stop=(k == nk - 1),
                    )
                nc.scalar.activation(
                    dst_tiles[m][:, fs], ps[:], func,
                    bias=bias_sb[:, bias_col_of(m, fi) : bias_col_of(m, fi) + 1],
                )

    both = [np.s_[ts(0, FREE)], np.s_[ts(1, FREE)]]
    # layer 1: h1 = relu(W1^T xT + b1)
    mm_layer(h1_sb, w1_sb, x_sb, both, lambda m, fi: _B1_COL + m, relu)
    # layer 2: h2 = relu(W2^T h1 + b2)
    mm_layer(h2_sb, w2_sb, h1_sb, both, lambda m, fi: _B2_COL + m, relu)
    # layer 3 (expert): per state s, h3[:, s] = relu(Kb1_s^T h2[:, s] + Bb1_s)
    for s in range(2):
        mm_layer(h3_sb, kb1_sb[s], h2_sb, [both[s]],
                 lambda m, fi, s=s: _BB1_COL + 4 * s + m, relu)
    # layer 4 (expert): out[:, s] = Kb2_s^T h3[:, s] + Bb2_s
    for s in range(2):
        mm_layer(out_sb, kb2_sb[s], h3_sb, [both[s]],
                 lambda m, fi, s=s: _BB2_COL + 2 * s + m, ident)

    for m in range(U2 // P):
        nc.sync.dma_start(out[ts(m, P), :], out_sb[m][:])


_COMPILED = None
LAST_RESULTS = None


def _get_program():
    global _COMPILED
    if _COMPILED is None:
        nc = bacc.Bacc("TRN2", target_bir_lowering=False, debug=False,
                       num_devices=NCORES)
        with tile.TileContext(nc) as tc:
            with ExitStack() as ctx:
                _body(tc, ctx)
        nc.compile()
        _COMPILED = nc
    return _COMPILED


def _route(states):
    """Return (counts, row_indices) where row_indices[k] are sample indices of
    state k in original order."""
    order = np.argsort(states, kind="stable")
    counts = np.bincount(states, minlength=K)
    starts = np.concatenate([[0], np.cumsum(counts)])
    rows = [order[starts[k]:starts[k + 1]] for k in range(K)]
    return counts, rows


def kernel(**inputs):
    x = np.ascontiguousarray(np.asarray(inputs["x"], dtype=np.float32))
    states = np.asarray(inputs["states"]).astype(np.int64)
    W1 = np.asarray(inputs["W1"], dtype=np.float32)
    b1 = np.asarray(inputs["b1"], dtype=np.float32)
    W2 = np.asarray(inputs["W2"], dtype=np.float32)
    b2 = np.asarray(inputs["b2"], dtype=np.float32)
    Kb1 = np.asarray(inputs["Kb1"], dtype=np.float32)
    Bb1 = np.asarray(inputs["Bb1"], dtype=np.float32)
    Kb2 = np.asarray(inputs["Kb2"], dtype=np.float32)
    Bb2 = np.asarray(inputs["Bb2"], dtype=np.float32)

    counts, rows = _route(states)
    assert counts.max() <= SCAP, f"state count {counts.max()} exceeds SCAP={SCAP}"

    shared_bias = np.zeros((P, 16), np.float32)
    for m in range(8):
        shared_bias[:, _B1_COL + m] = b1[m * P:(m + 1) * P]
        shared_bias[:, _B2_COL + m] = b2[m * P:(m + 1) * P]

    in_maps = []
    for c in range(NCORES):
        sa, sb = 2 * c, 2 * c + 1
        xr = np.zeros((R, F), np.float32)
        xr[0:counts[sa]] = x[rows[sa]]
        xr[SCAP:SCAP + counts[sb]] = x[rows[sb]]
        bias = np.zeros((P, _NBIAS), np.float32)
        bias[:, :16] = shared_bias
        for s, st in enumerate((sa, sb)):
            for m in range(4):
                bias[:, _BB1_COL + 4 * s + m] = Bb1[0, m * P:(m + 1) * P, st]
            for m in range(2):
                bias[:, _BB2_COL + 2 * s + m] = Bb2[0, m * P:(m + 1) * P, st]
        in_maps.append({
            "xT": np.ascontiguousarray(xr.T),
            "w1": W1,
            "w2": W2,
            "kb1": np.ascontiguousarray(
                np.stack([Kb1[:, :, sa], Kb1[:, :, sb]])),
            "kb2": np.ascontiguousarray(
                np.stack([Kb2[:, :, sa], Kb2[:, :, sb]])),
            "biases": bias,
        })

    nc = _get_program()
    res = run_bass_kernel_spmd(nc, in_maps, core_ids=list(range(NCORES)))
    global LAST_RESULTS
    LAST_RESULTS = res

    out = np.zeros((B, U2), np.float32)
    for c in range(NCORES):
        o = res.results[c]["out"]  # [U2, R]
        sa, sb = 2 * c, 2 * c + 1
        out[rows[sa]] = o[:, 0:counts[sa]].T
        out[rows[sb]] = o[:, SCAP:SCAP + counts[sb]].T
    return out
